# revision 1
# baseline (speedup 1.0000x reference)
"""Trainium2 Bass kernel for the ActorCriticSNN LIF network (DSQN drone).

Strategy (data-parallel over batch, 16 elements per core, 8 cores):
  Normalized coords u = (mem - thr)/thr make the LIF step
      u_t = beta*u_{t-1} - (u_{t-1} > 0) + c_t,   spike s_t = (u_t > 0)
  with u_init = -1.  Each tick is 4 DVE instructions, one [128, 64] pair
  per layer, ordered L1,L2,L1,L2 so every RAW dependency has reuse
  distance 2 (hides the SBUF write->read turnaround):
      op1 (custom DVE op LIF_DECAY_RESET): r = u*beta - (u > 0)
      op2 (tensor_tensor add):             u' = r + c
  beta rides as a full stream (per-partition scalar operands cost ~120ns
  extra per instruction).  u' goes into a 4-block fused history ring
  [128, (t, layer, group, batch)]; ScalarE extracts spikes as Sign(u')
  in {-1,+1} fp16 once per 8-tick block per layer half, feeding TensorE
  GEMMs with single-fp16 folded weights (rel err ~5e-3, tol 2e-2).
  Layer 2 lags layer 1 by D=16 ticks; its input c2 = W2 @ spk1 is
  written by the GEMM epilogue into a global-tick-indexed stream.  Both
  layers' c streams are contiguous in SBUF so all DMA is descriptor-
  friendly.  Frozen edge ticks (L2 before D, L1 after T) are skipped.
  The action layer accumulates via tensor_tensor_scan, in three chunks.
"""
import sys
import numpy as np

sys.path.insert(0, '/opt/trn_rl_repo')

import concourse.bass as bass  # noqa: E402
import concourse.tile as tile  # noqa: E402
from concourse import bacc, mybir  # noqa: E402
from concourse.bass_utils import run_bass_kernel_spmd  # noqa: E402
from concourse.dve_spec import Spec, Src0, Src1, Zero, lower  # noqa: E402
from concourse import dve_ops as dve_ops_mod  # noqa: E402
from concourse.dve_ops import DveOp, DveOpSpec, OPS, CUSTOM_DVE_SPECS  # noqa: E402

# Problem constants (hardcoded per spec)
B, T, NIN, H, NACT = 128, 256, 16, 512, 4
N_CORES = 8
BL = B // N_CORES          # 16 batch per core
SC = 4 * BL                # per-layer step columns (4 feature groups x 16)
SC2 = 2 * SC               # both layers fused in hist (128)
TB = 8                     # ticks per block (sign/GEMM granularity)
D = 16                     # layer-2 lag in ticks (multiple of TB)
NTICK = T + D              # 272
NB = NTICK // TB           # 34 hist/sign blocks
HB = 4                     # history ring blocks
HSLOTS = HB * TB           # 32 tick slots in the ring
SP2 = SC2                  # hist slot stride (pad tested neutral, kept at 0)
NQ = 8                     # c1 DMA chunks
QS = NTICK // NQ           # 34 ticks per chunk

_cache = {}


def _register_lif_op():
    name = "LIF_DECAY_RESET"
    for op in OPS:
        if op.name == name:
            return op
    spec = Spec(
        body=Src0 * Src1 - (Src0 > Zero),
        reference=lambda in0, in1: in0 * in1 - (in0 > 0).astype(in0.dtype),
    )
    shas = {}
    for ver in ("v3", "v4"):
        uops = lower(spec, ver=ver)
        shas[ver] = DveOpSpec(name=name, opcode=1, uops=uops, rd1_en=True).sha(ver)
    op = DveOp(name, spec, subdim=False, uops_sha=shas)
    OPS.append(op)
    dve_ops_mod._SUB_OPCODE_FOR_NAME[name] = (
        dve_ops_mod._CUSTOM_DVE_ROW_BASE + len(OPS) - 1
    )
    CUSTOM_DVE_SPECS[name] = spec
    return op


def _build_program():
    """Build the per-core Bass program (same NEFF on all 8 cores)."""
    lif_op = _register_lif_op()
    fp32 = mybir.dt.float32
    fp16 = mybir.dt.float16
    Sign = mybir.ActivationFunctionType.Sign
    Ident = mybir.ActivationFunctionType.Identity
    Op = mybir.AluOpType

    nc = bacc.Bacc("TRN2", target_bir_lowering=False, debug=False,
                   num_devices=N_CORES)

    # ---- DRAM parameters ----
    c1q_e = [nc.dram_tensor(f"c1q{i}", [128, QS * SC], fp32,
                            kind="ExternalInput").ap() for i in range(NQ)]
    w2_e = nc.dram_tensor("w2", [128, 16 * 128], fp16, kind="ExternalInput").ap()
    wa_e = nc.dram_tensor("wa", [128, 16], fp16, kind="ExternalInput").ap()
    b2n_e = nc.dram_tensor("b2n", [128, 4], fp32, kind="ExternalInput").ap()
    ban_e = nc.dram_tensor("ban", [NACT, 1], fp32, kind="ExternalInput").ap()
    bt12_e = nc.dram_tensor("bt12", [128, SC2], fp32, kind="ExternalInput").ap()
    out_e = nc.dram_tensor("out", [4 * BL, T], fp32, kind="ExternalOutput").ap()

    with tile.TileContext(nc) as tc:
        import contextlib
        with contextlib.ExitStack() as ctx:
            consts = ctx.enter_context(tc.tile_pool(name="consts", bufs=1))
            s12p = ctx.enter_context(tc.tile_pool(name="s12p", bufs=3))
            ps2p = ctx.enter_context(tc.tile_pool(name="ps2p", bufs=5, space="PSUM"))
            ps3p = ctx.enter_context(tc.tile_pool(name="ps3p", bufs=3, space="PSUM"))

            # ---- tiles ----
            c1sb = consts.tile([128, NTICK * SC], fp32, name="c1sb")
            c2sb = consts.tile([128, NTICK * SC], fp32, name="c2sb")
            c2v = c2sb.rearrange("p (t c) -> p t c", c=SC)
            w2 = consts.tile([128, 16 * 128], fp16)
            wa = consts.tile([128, 16], fp16)
            b2n = consts.tile([128, 4], fp32)
            ban = consts.tile([NACT, 1], fp32)
            bt12 = consts.tile([128, SC2], fp32)
            hist = consts.tile([128, HSLOTS * SP2], fp32, name="hist")
            r1 = consts.tile([128, SC], fp32, name="r1")
            r2 = consts.tile([128, SC], fp32, name="r2")
            act_arr = consts.tile([NACT, BL * T], fp32)     # col = b*T + t
            act64 = consts.tile([4 * BL, T], fp32)          # part = a*BL + b
            decay = consts.tile([4 * BL, T], fp32)
            out_sb = consts.tile([4 * BL, T], fp32)

            # ---- load constants; order = dependency priority ----
            for dst, src in [(bt12, bt12_e), (b2n, b2n_e), (ban, ban_e)]:
                nc.sync.dma_start(out=dst, in_=src)
            # progressive contiguous c1 loads
            def c1_load(q, lo, hi):
                nc.sync.dma_start(
                    out=c1sb[:, (q * QS + lo) * SC:(q * QS + hi) * SC],
                    in_=c1q_e[q][:, lo * SC:hi * SC])
            for lo_s, hi_s in ((0, 1), (1, 8), (8, QS)):
                c1_load(0, lo_s, hi_s)
            for dst, src in [(w2, w2_e), (wa, wa_e)]:
                nc.sync.dma_start(out=dst, in_=src)
            for q in range(1, NQ):
                c1_load(q, 0, QS)

            # init: u(-1) = -1 in ring slot HSLOTS-1 (L1 half);
            # u2 state enters at tick D reading slot D-1 (L2 half)
            nc.vector.memset(hist[:, (HSLOTS - 1) * SP2:(HSLOTS - 1) * SP2 + SC], -1.0)
            nc.vector.memset(
                hist[:, (D - 1) * SP2 + SC:(D - 1) * SP2 + SC2], -1.0)
            nc.vector.memset(decay, 0.95)
            nc.vector.memset(decay[:, 0:1], 0.0)

            # trigger ACT table load early, overlapped with input DMAs
            actwarm = consts.tile([4, 1], fp32)
            nc.vector.memset(actwarm, 0.0)
            nc.scalar.activation(out=actwarm, in_=actwarm, func=Sign)

            s12_blocks = {}

            def sign_block(kb):
                """Spikes for hist block kb as Sign(u) in {-1,+1} fp16.
                Split per layer half: L1 feeds g2 (blocks <= 31), L2 feeds
                g3 (blocks >= D/TB)."""
                off = (kb % HB) * TB * SP2
                hv = hist[:, off:off + TB * SP2].rearrange(
                    "p (t c) -> p t c", c=SP2)[:, :, 0:SC2]
                sblk = s12p.tile([128, TB * SC2], fp16, name=f"s12b{kb}",
                                 tag="s12roll")
                s12_blocks[kb] = sblk
                sv = sblk.rearrange("p (t c) -> p t c", c=SC2)
                if kb <= 31:
                    nc.scalar.activation(out=sv[:, :, 0:SC],
                                         in_=hv[:, :, 0:SC], func=Sign)
                if kb >= D // TB:
                    nc.scalar.activation(out=sv[:, :, SC:SC2],
                                         in_=hv[:, :, SC:SC2], func=Sign)

            def g2_block(m):
                """c2 for L1-time block m -> c2 stream at ticks m*8+D..+8."""
                sblk = s12_blocks[m]
                srear = sblk.rearrange("p (t c) -> p t c", c=SC2)
                t0 = m * TB + D
                for mq in range(4):
                    ps = ps2p.tile([128, TB * BL], fp32, name=f"ps2_{m}_{mq}",
                                   tag="ps2")
                    for j in range(4):
                        nc.tensor.matmul(
                            ps,
                            w2[:, (j * 4 + mq) * 128:(j * 4 + mq + 1) * 128],
                            srear[:, :, j * BL:(j + 1) * BL],
                            start=(j == 0), stop=(j == 3))
                    nc.scalar.activation(
                        out=c2v[:, t0:t0 + TB, mq * BL:(mq + 1) * BL],
                        in_=ps.rearrange("p (t b) -> p t b", b=BL),
                        func=Ident, bias=b2n[:, mq:mq + 1], scale=1.0)

            def g3_block(m):
                """act for times m*8..m*8+8 from L2 spikes (hist block m+D/TB)."""
                sblk = s12_blocks[m + D // TB]
                srear = sblk.rearrange("p (t c) -> p t c", c=SC2)
                ps = ps3p.tile([NACT, TB * BL], fp32, name=f"ps3_{m}", tag="ps3")
                for j in range(4):
                    nc.tensor.matmul(
                        ps,
                        wa[:, j * 4:(j + 1) * 4],
                        srear[:, :, SC + j * BL:SC + (j + 1) * BL],
                        start=(j == 0), stop=(j == 3))
                nc.scalar.activation(
                    out=act_arr.rearrange("p (b t) -> p b t", t=T)[:, :, m * TB:(m + 1) * TB],
                    in_=ps.rearrange("p (t b) -> p b t", b=BL),
                    func=Ident, bias=ban, scale=1.0)

            def act_fetch(lo, hi):
                nc.sync.dma_start(
                    out=act64[:, lo:hi],
                    in_=act_arr.rearrange("p (b t) -> p b t", t=T)[:, :, lo:hi])

            def act_scan(lo, hi, first):
                nc.vector.tensor_tensor_scan(
                    out=out_sb[:, lo:hi], data0=decay[:, lo:hi],
                    data1=act64[:, lo:hi],
                    initial=0.0 if first else out_sb[:, lo - 1:lo],
                    op0=Op.mult, op1=Op.add)
                nc.sync.dma_start(out=out_e[:, lo:hi], in_=out_sb[:, lo:hi])

            # ---- main tick loop ----
            for tk in range(NTICK):
                if tk % TB == 0:
                    kb = tk // TB
                    if kb >= 1:
                        sign_block(kb - 1)
                    if 1 <= kb <= 32:
                        g2_block(kb - 1)
                    if 3 <= kb <= 33:
                        g3_block(kb - 3)
                    if kb == 20:
                        act_fetch(0, 128)            # g3 done through m=17
                    if kb == 21:
                        act_scan(0, 128, True)
                    if kb == 28:
                        act_fetch(128, 192)          # g3 done through m=25
                    if kb == 29:
                        act_scan(128, 192, False)
                    if kb == 33:
                        act_fetch(192, 240)          # g3 done through m=29
                sl_prev = ((tk - 1) % HSLOTS) * SP2
                sl = (tk % HSLOTS) * SP2
                l1_on = tk < T
                l2_on = tk >= D
                # interleave independent chains so every RAW dependency
                # lands at reuse distance 2: the two layers when both are
                # active, otherwise the active layer's two batch halves
                if l1_on and l2_on:
                    nc.vector._custom_dve(
                        lif_op, out=r1,
                        in0=hist[:, sl_prev:sl_prev + SC], in1=bt12[:, 0:SC])
                    nc.vector._custom_dve(
                        lif_op, out=r2,
                        in0=hist[:, sl_prev + SC:sl_prev + SC2],
                        in1=bt12[:, SC:SC2])
                    nc.vector.tensor_tensor(
                        out=hist[:, sl:sl + SC], in0=r1,
                        in1=c1sb[:, tk * SC:(tk + 1) * SC], op=Op.add)
                    nc.vector.tensor_tensor(
                        out=hist[:, sl + SC:sl + SC2], in0=r2,
                        in1=c2sb[:, tk * SC:(tk + 1) * SC], op=Op.add)
                else:
                    off = 0 if l1_on else SC
                    csb = c1sb if l1_on else c2sb
                    r = r1 if l1_on else r2
                    HS = SC // 2
                    for h in range(2):
                        nc.vector._custom_dve(
                            lif_op, out=r[:, h * HS:(h + 1) * HS],
                            in0=hist[:, sl_prev + off + h * HS:
                                     sl_prev + off + (h + 1) * HS],
                            in1=bt12[:, off + h * HS:off + (h + 1) * HS])
                    for h in range(2):
                        nc.vector.tensor_tensor(
                            out=hist[:, sl + off + h * HS:sl + off + (h + 1) * HS],
                            in0=r[:, h * HS:(h + 1) * HS],
                            in1=csb[:, tk * SC + h * HS:tk * SC + (h + 1) * HS],
                            op=Op.add)

            # ---- tail ----
            sign_block(NB - 1)
            g3_block(31)
            act_scan(192, 240, False)
            act_fetch(240, T)
            act_scan(240, T, False)

    nc.compile()
    return nc


def _prep_inputs(inputs):
    """Host-side prep: normalized fp16 weights + per-core c1 streams."""
    x = np.asarray(inputs["batch"], np.float32)        # [B, T, NIN]
    W1 = np.asarray(inputs["W1"], np.float32); b1 = np.asarray(inputs["b1"], np.float32)
    W2 = np.asarray(inputs["W2"], np.float32); b2 = np.asarray(inputs["b2"], np.float32)
    Wa = np.asarray(inputs["Wa"], np.float32); ba = np.asarray(inputs["ba"], np.float32)
    beta1 = np.clip(np.asarray(inputs["beta1"], np.float32), 0, 1)
    thr1 = np.asarray(inputs["thr1"], np.float32)
    beta2 = np.clip(np.asarray(inputs["beta2"], np.float32), 0, 1)
    thr2 = np.asarray(inputs["thr2"], np.float32)
    mn = float(np.float32(inputs["inp_min"])); mx = float(np.float32(inputs["inp_max"]))
    R = mx - mn

    W1n = (W1 / R) / thr1[:, None]
    b1n = (b1 - (mn / R) * W1.sum(1)) / thr1 + beta1 - 1.0

    # +-1 spike encoding folded into single-fp16 weights
    W2n = W2 / thr2[:, None]
    b2n = b2 / thr2 + beta2 - 1.0
    W2e = (W2n / 2).astype(np.float16)
    b2tot = b2n + W2e.astype(np.float32).sum(1)
    Wae = (Wa / 2).astype(np.float16)
    batot = ba + Wae.astype(np.float32).sum(1)

    def chunked_w2(w):  # [512,512] -> W2eT chunk layout: col (j*4+m)*128 + mc
        wt = np.asarray(w).T
        outw = np.zeros((128, 16 * 128), w.dtype)
        for j in range(4):
            for m in range(4):
                outw[:, (j * 4 + m) * 128:(j * 4 + m + 1) * 128] = \
                    wt[j * 128:(j + 1) * 128, m * 128:(m + 1) * 128]
        return outw

    def chunked_wa(w):  # [4,512] -> WaeT chunks: col j*4 + a
        wt = np.asarray(w).T
        outw = np.zeros((128, 16), w.dtype)
        for j in range(4):
            outw[:, j * 4:(j + 1) * 4] = wt[j * 128:(j + 1) * 128, :]
        return outw

    def beta_tile(beta):
        return np.ascontiguousarray(
            np.repeat(beta.reshape(4, 128).T[:, :, None], BL, 2).reshape(128, SC))

    bt1 = beta_tile(beta1)
    bt2 = beta_tile(beta2)
    common = {
        "w2": np.ascontiguousarray(chunked_w2(W2e)),
        "wa": np.ascontiguousarray(chunked_wa(Wae)),
        "b2n": np.ascontiguousarray(b2tot.reshape(4, 128).T.astype(np.float32)),
        "ban": np.ascontiguousarray(batot.reshape(NACT, 1).astype(np.float32)),
        "bt12": np.ascontiguousarray(np.concatenate([bt1, bt2], 1)),
    }

    # per-core c1 = W1n @ x_t + b1n (fp32), tick-major [128, t*SC + g*BL + b],
    # padded arbitrarily (frozen values) for ticks T..NTICK-1 (never consumed)
    frz1 = np.broadcast_to((bt1 - 1.0)[:, None, :],
                           (128, NTICK - T, SC)).reshape(128, (NTICK - T) * SC)
    xt = x.transpose(1, 0, 2)  # [T, B, NIN]
    in_maps = []
    for c in range(N_CORES):
        xs = xt[:, c * BL:(c + 1) * BL, :]                    # [T, BL, NIN]
        c1 = np.einsum('hk,tbk->thb', W1n, xs).astype(np.float32) \
            + b1n[None, :, None]                              # [T, 512, BL]
        c1c = np.ascontiguousarray(
            c1.reshape(T, 4, 128, BL).transpose(2, 0, 1, 3)
            .reshape(128, T * SC))
        c1full = np.concatenate([c1c, frz1], 1)               # [128, NTICK*SC]
        m = dict(common)
        for i in range(NQ):
            m[f"c1q{i}"] = np.ascontiguousarray(
                c1full[:, i * QS * SC:(i + 1) * QS * SC])
        in_maps.append(m)
    return in_maps


def _get_nc():
    if "nc" not in _cache:
        _cache["nc"] = _build_program()
    return _cache["nc"]


def _run(inputs, trace=False, trace_kwargs=None):
    nc = _get_nc()
    in_maps = _prep_inputs(inputs)
    res = run_bass_kernel_spmd(nc, in_maps, core_ids=list(range(N_CORES)),
                               trace=trace, **(trace_kwargs or {}))
    outs = []
    for c in range(N_CORES):
        o = np.asarray(res.results[c]["out"], np.float32)  # [(a,b), t]
        outs.append(o.reshape(NACT, BL, T).transpose(2, 1, 0))  # [T, BL, 4]
    full = np.concatenate(outs, axis=1)          # [T, B, 4]
    return full.reshape(1, T, B * NACT).astype(np.float32), res


def kernel(**inputs) -> np.ndarray:
    out, _ = _run(inputs, trace=False)
    return out

